# revision 19
# baseline (speedup 1.0000x reference)
"""Trainium2 Bass kernel for dot-product attention with q_len=1.

Reference computation (per batch b):
    q = query @ WQ^T            [1, H]
    k = key @ WK^T              [S, H]
    v = value @ WV^T            [S, H]
    scores = q @ k^T / sqrt(H)  [1, S]
    scores = where(mask, -1e15, scores)
    p = softmax(scores)         [1, S]
    out = p @ v                 [1, H]
    returns (out, p)

Algebraic restructuring used here (exact same math, fp32 throughout):
    scores = (query @ WQ^T @ WK) @ key^T / sqrt(H)  -> fold both projections
             into a single per-batch vector q_tilde, never materialize k.
    out    = (p @ value) @ WV^T                     -> never materialize v.
This removes the two [S,H]x[H,H] GEMMs entirely; the kernel is then purely
memory-bound on streaming key+value (64 MiB/core) once, which is the roofline.

Work split on-chip: scores are computed on the VectorE with fused
tensor_tensor_reduce (key stays in its natural [s, d] layout — contraction
along the free dim), while the p@value contraction runs on the TensorE
(contraction along partitions).  fp32 matmuls cost 4 cycles/row on the PE,
so keeping the big key contraction off the PE is what makes both engines
fit under the DMA roofline.

Distribution: data-parallel over the batch dim, 8 batches per core on
8 cores.  The s index maps to (partition p, column j) as s = p*32 + j so
key/value/mask/p_attn transfers are all fully contiguous per partition.
"""

from contextlib import ExitStack

import numpy as np

B, S, H = 64, 4096, 256
NCORES = 8
BPC = B // NCORES  # batches per core
SJ = 32  # s-chunk columns: s = p*32 + j, p in [0,128), j in [0,32)
SCALE = 1.0 / 16.0  # 1/sqrt(H)
XPE = 12  # u-pass j-columns handled by the TensorE; the rest go to DVE
NEG = -1.0e15

_NC_CACHE = {}


def build_nc():
    import concourse.bacc as bacc
    import concourse.bass as bass
    import concourse.mybir as mybir
    import concourse.tile as tile

    f32 = mybir.dt.float32
    u8 = mybir.dt.uint8
    ts = bass.ts

    nc = bacc.Bacc("TRN2", target_bir_lowering=False, debug=False)

    queryT = nc.dram_tensor("queryT", [H, BPC], f32, kind="ExternalInput")[:]
    key = nc.dram_tensor("key", [BPC, S, H], f32, kind="ExternalInput")[:]
    value = nc.dram_tensor("value", [BPC, S, H], f32, kind="ExternalInput")[:]
    mask = nc.dram_tensor("mask", [BPC, S], u8, kind="ExternalInput")[:]
    WQT = nc.dram_tensor("WQT", [H, H], f32, kind="ExternalInput")[:]
    WK = nc.dram_tensor("WK", [H, H], f32, kind="ExternalInput")[:]
    WVT = nc.dram_tensor("WVT", [H, H], f32, kind="ExternalInput")[:]
    ident128 = nc.dram_tensor("ident128", [128, 128], f32, kind="ExternalInput")[:]
    out = nc.dram_tensor("out", [BPC, H], f32, kind="ExternalOutput")[:]
    pattn = nc.dram_tensor("pattn", [BPC, S], f32, kind="ExternalOutput")[:]

    with tile.TileContext(nc) as tc, ExitStack() as ctx:
        singles = ctx.enter_context(tc.tile_pool(name="singles", bufs=1))
        kpool = ctx.enter_context(tc.tile_pool(name="key", bufs=4))
        vpool = ctx.enter_context(tc.tile_pool(name="val", bufs=4))
        spool = ctx.enter_context(tc.tile_pool(name="small", bufs=3))
        jpool = ctx.enter_context(tc.tile_pool(name="junk", bufs=2))
        ps_u = ctx.enter_context(tc.tile_pool(name="ps_u", bufs=2, space="PSUM"))
        ps_t = ctx.enter_context(tc.tile_pool(name="ps_t", bufs=4, space="PSUM"))
        ascr_pool = ctx.enter_context(tc.tile_pool(name="ascr", bufs=2))
        acc_pool = ctx.enter_context(tc.tile_pool(name="acc", bufs=2))

        # ---- constants ----
        ones_col = singles.tile([128, 1], f32)
        nc.vector.memset(ones_col[:], 1.0)
        ones_row = singles.tile([1, 128], f32)
        nc.vector.memset(ones_row[:], 1.0)
        ident = singles.tile([128, 128], f32)
        nc.sync.dma_start(ident[:], ident128)

        # ---- load weights / query (tiny) ----
        wqt = singles.tile([128, 2, H], f32)  # [d_p, dc, e] = WQ^T
        nc.sync.dma_start(wqt[:], WQT.rearrange("(dc p) e -> p dc e", p=128))
        wk = singles.tile([128, 2, H], f32)  # [e_p, ec, d] = WK (natural)
        nc.sync.dma_start(wk[:], WK.rearrange("(ec p) d -> p ec d", p=128))
        wvt = singles.tile([128, 2, H], f32)  # [d_p, dc, e] = WV^T
        nc.sync.dma_start(wvt[:], WVT.rearrange("(dc p) e -> p dc e", p=128))
        qt = singles.tile([128, 2, BPC], f32)  # [d_p, dc, b] = query^T
        nc.sync.dma_start(qt[:], queryT.rearrange("(dc p) b -> p dc b", p=128))

        # ---- q^T = WQ @ query^T : [e, b] ----
        q_sb = singles.tile([128, 2, BPC], f32)
        with tc.tile_pool(name="ps_setup", bufs=1, space="PSUM") as ps_set:
            for ec in range(2):
                qp = ps_set.tile([128, BPC], f32, tag="setup")
                for dc in range(2):
                    nc.tensor.matmul(
                        qp[:],
                        wqt[:, dc, ts(ec, 128)],
                        qt[:, dc, :],
                        start=(dc == 0),
                        stop=(dc == 1),
                    )
                nc.vector.tensor_copy(q_sb[:, ec, :], qp[:])
            # ---- q_tilde^T = (WK^T @ q^T) * SCALE : [d_p, dc, b] ----
            qtil = singles.tile([128, 2, BPC], f32)
            for dc in range(2):
                qtp = ps_set.tile([128, BPC], f32, tag="setup")
                for ec in range(2):
                    nc.tensor.matmul(
                        qtp[:],
                        wk[:, ec, ts(dc, 128)],
                        q_sb[:, ec, :],
                        start=(ec == 0),
                        stop=(ec == 1),
                    )
                nc.scalar.mul(qtil[:, dc, :], qtp[:], SCALE)

            # ---- broadcast every batch's q_tilde to all partitions ----
            # qball[:, b, :] = q_tilde_b replicated on 128 partitions
            qball = singles.tile([128, BPC, H], f32)
            for b in range(BPC):
                qrow = spool.tile([1, H], f32)
                for dc in range(2):
                    qrp = ps_t.tile([1, 128], f32, tag="tiny")
                    nc.tensor.transpose(qrp[:], qtil[:, dc, b : b + 1], ident[:])
                    nc.vector.tensor_copy(qrow[0:1, ts(dc, 128)], qrp[:])
                qbp = ps_set.tile([128, H], f32, tag="qb")
                nc.tensor.matmul(qbp[:], ones_row[:], qrow[:])
                nc.vector.tensor_copy(qball[:, b, :], qbp[:])

        # ---- per-batch attention ----
        HJ = SJ // 2  # half of the j range, for finer DMA/compute pipelining
        for b in range(BPC):
            key_b = key[b].rearrange("(p j) d -> p j d", p=128)
            val_b = value[b].rearrange("(p j) d -> p j d", p=128)
            kts = []
            vts = []
            for h in range(2):
                kt = kpool.tile([128, HJ, H], f32, tag="kt")  # s = p*32+j
                nc.sync.dma_start(kt[:], key_b[:, h * HJ : (h + 1) * HJ, :])
                kts.append(kt)
            for h in range(2):
                vt = vpool.tile([128, HJ, H], f32, tag="vt")
                nc.sync.dma_start(vt[:], val_b[:, h * HJ : (h + 1) * HJ, :])
                vts.append(vt)
            msk = spool.tile([128, SJ], u8)
            nc.sync.dma_start(msk[:], mask[b].rearrange("(p j) -> p j", p=128))
            mskf = spool.tile([128, SJ], f32)
            nc.vector.tensor_copy(mskf[:], msk[:])  # u8 -> f32 cast

            # scores[s] = q_tilde . key[s]: elementwise mult on DVE
            # (q_tilde broadcast along j), then per-j reduction on ScalarE
            # via activation(Identity, accum_out) to keep DVE free
            scs = spool.tile([128, SJ], f32)
            qb_bc = qball[:, b : b + 1, :].broadcast_to([128, HJ, H])
            for h in range(2):
                junk = jpool.tile([128, HJ, H], f32, tag="junk")
                nc.vector.tensor_tensor(
                    junk[:], kts[h][:], qb_bc, op=mybir.AluOpType.mult
                )
                for j in range(HJ):
                    jj = h * HJ + j
                    ascr = ascr_pool.tile([128, H], f32, tag="ascr")
                    nc.scalar.activation(
                        ascr[:],
                        junk[:, j, :],
                        mybir.ActivationFunctionType.Identity,
                        accum_out=scs[:, jj : jj + 1],
                    )

            # additive mask, then exp (fused with row-sum accumulation)
            e = spool.tile([128, SJ], f32)
            rs = spool.tile([128, 1], f32)
            nc.vector.tensor_scalar(
                e[:], mskf[:], NEG, None, op0=mybir.AluOpType.mult
            )
            nc.vector.tensor_tensor(e[:], scs[:], e[:], op=mybir.AluOpType.add)
            nc.scalar.activation(
                e[:], e[:], mybir.ActivationFunctionType.Exp, accum_out=rs[:]
            )

            # Z = sum over partitions; zi = 1/Z
            zp = ps_t.tile([1, 1], f32, tag="tiny")
            nc.tensor.matmul(zp[:], rs[:], ones_col[:])
            zi = spool.tile([1, 1], f32)
            nc.vector.reciprocal(zi[:], zp[:])

            # u' = e @ value (unnormalized), split PE/DVE to balance load:
            # j < XPE on the TensorE (contraction over partitions),
            # j >= XPE accumulated on DVE then partition-reduced on the PE
            up = ps_u.tile([1, H], f32, tag="uo")
            for j in range(XPE):
                nc.tensor.matmul(
                    up[:],
                    e[:, j : j + 1],
                    vts[j // HJ][:, j % HJ, :],
                    start=(j == 0),
                    stop=(j == XPE - 1),
                )
            acc = acc_pool.tile([128, H], f32, tag="acc")
            tmp = acc_pool.tile([128, H], f32, tag="tmp")
            for j in range(XPE, SJ):
                vtj = vts[j // HJ][:, j % HJ, :]
                if j == XPE:
                    nc.vector.tensor_scalar_mul(acc[:], vtj, e[:, j : j + 1])
                else:
                    nc.vector.tensor_scalar_mul(tmp[:], vtj, e[:, j : j + 1])
                    nc.vector.tensor_tensor(
                        acc[:], acc[:], tmp[:], op=mybir.AluOpType.add
                    )

            # p_attn = e * (1/Z) broadcast — off the critical path
            zbp = ps_t.tile([128, 1], f32, tag="tiny")
            nc.tensor.matmul(zbp[:], ones_row[:], zi[:])
            zb = spool.tile([128, 1], f32)
            nc.vector.tensor_copy(zb[:], zbp[:])
            probs = spool.tile([128, SJ], f32)
            nc.vector.tensor_scalar_mul(probs[:], e[:], zb[:])
            nc.scalar.dma_start(pattn[b].rearrange("(p j) -> p j", p=128), probs[:])

            # combine the two u' partials as u^T [128, 2], normalize, project
            up_sb = spool.tile([1, H], f32)
            nc.vector.tensor_copy(up_sb[:], up[:])
            ut = spool.tile([128, 2], f32)
            for dc in range(2):
                utp = ps_t.tile([128, 1], f32, tag="tiny")
                nc.tensor.transpose(utp[:], up_sb[0:1, ts(dc, 128)], ident[0:1, 0:1])
                nc.vector.tensor_copy(ut[:, dc : dc + 1], utp[:])
            utd = spool.tile([128, 2], f32)
            for dc in range(2):
                udp = ps_t.tile([128, 1], f32, tag="tiny")
                nc.tensor.matmul(udp[:], acc[:, ts(dc, 128)], ones_col[:])
                nc.vector.tensor_copy(utd[:, dc : dc + 1], udp[:])
            nc.vector.tensor_tensor(ut[:], ut[:], utd[:], op=mybir.AluOpType.add)
            nc.vector.tensor_scalar_mul(ut[:], ut[:], zb[:])
            op_ = ps_u.tile([1, H], f32, tag="uo")
            for dc in range(2):
                nc.tensor.matmul(
                    op_[:],
                    ut[:, dc : dc + 1],
                    wvt[:, dc, :],
                    start=(dc == 0),
                    stop=(dc == 1),
                )
            ob = spool.tile([1, H], f32)
            nc.scalar.copy(ob[:], op_[:])
            nc.scalar.dma_start(out[b : b + 1, :], ob[:])

    nc.compile()
    return nc


def _get_nc():
    if "nc" not in _NC_CACHE:
        _NC_CACHE["nc"] = build_nc()
    return _NC_CACHE["nc"]


def make_in_maps(query, key, value, mask):
    """Shard + lay out the full inputs for the 8 cores (no arithmetic)."""
    query = np.asarray(query, dtype=np.float32).reshape(B, H)
    key = np.asarray(key, dtype=np.float32)
    value = np.asarray(value, dtype=np.float32)
    mask_u8 = np.ascontiguousarray(np.asarray(mask)).view(np.uint8)
    ident = np.eye(128, dtype=np.float32)

    in_maps = []
    for c in range(NCORES):
        sl = slice(c * BPC, (c + 1) * BPC)
        q = query[sl]  # [BPC, H]
        in_maps.append(
            {
                "queryT": np.ascontiguousarray(q.T),
                "key": np.ascontiguousarray(key[sl]),
                "value": np.ascontiguousarray(value[sl]),
                "mask": np.ascontiguousarray(mask_u8[sl]),
                "ident128": ident,
            }
        )
    return in_maps


def _add_weights(in_maps, WQ, WK, WV):
    WQT = np.ascontiguousarray(np.asarray(WQ, dtype=np.float32).T)
    WKc = np.ascontiguousarray(np.asarray(WK, dtype=np.float32))
    WVT = np.ascontiguousarray(np.asarray(WV, dtype=np.float32).T)
    for m in in_maps:
        m["WQT"] = WQT
        m["WK"] = WKc
        m["WVT"] = WVT
    return in_maps


def run(query, key, value, mask, WQ, WK, WV, trace=False, **spmd_kwargs):
    from concourse.bass_utils import run_bass_kernel_spmd

    nc = _get_nc()
    in_maps = _add_weights(make_in_maps(query, key, value, mask), WQ, WK, WV)
    res = run_bass_kernel_spmd(
        nc, in_maps, list(range(NCORES)), trace=trace, **spmd_kwargs
    )
    outs = np.concatenate([res.results[c]["out"] for c in range(NCORES)], axis=0)
    patt = np.concatenate([res.results[c]["pattn"] for c in range(NCORES)], axis=0)
    return outs.reshape(B, 1, H), patt.reshape(B, 1, S), res


def kernel(query, key, value, mask, WQ, WK, WV):
    outs, patt, _ = run(query, key, value, mask, WQ, WK, WV)
    return outs, patt


# revision 20
# speedup vs baseline: 1.1953x; 1.1953x over previous
"""Trainium2 Bass kernel for dot-product attention with q_len=1.

Reference computation (per batch b):
    q = query @ WQ^T            [1, H]
    k = key @ WK^T              [S, H]
    v = value @ WV^T            [S, H]
    scores = q @ k^T / sqrt(H)  [1, S]
    scores = where(mask, -1e15, scores)
    p = softmax(scores)         [1, S]
    out = p @ v                 [1, H]
    returns (out, p)

Algebraic restructuring used here (exact same math, fp32 throughout):
    scores = (query @ WQ^T @ WK) @ key^T / sqrt(H)  -> fold both projections
             into a single per-batch vector q_tilde, never materialize k.
    out    = (p @ value) @ WV^T                     -> never materialize v.
This removes the two [S,H]x[H,H] GEMMs entirely; the kernel is then purely
memory-bound on streaming key+value (64 MiB/core) once, which is the roofline.

Work split on-chip: scores are computed on the VectorE with fused
tensor_tensor_reduce (key stays in its natural [s, d] layout — contraction
along the free dim), while the p@value contraction runs on the TensorE
(contraction along partitions).  fp32 matmuls cost 4 cycles/row on the PE,
so keeping the big key contraction off the PE is what makes both engines
fit under the DMA roofline.

Distribution: data-parallel over the batch dim, 8 batches per core on
8 cores.  The s index maps to (partition p, column j) as s = p*32 + j so
key/value/mask/p_attn transfers are all fully contiguous per partition.
"""

from contextlib import ExitStack

import numpy as np

B, S, H = 64, 4096, 256
NCORES = 8
BPC = B // NCORES  # batches per core
SJ = 32  # s-chunk columns: s = p*32 + j, p in [0,128), j in [0,32)
SCALE = 1.0 / 16.0  # 1/sqrt(H)
XPE = 12  # u-pass j-columns handled by the TensorE; the rest go to DVE
NEG = -1.0e15

_NC_CACHE = {}


def build_nc():
    import concourse.bacc as bacc
    import concourse.bass as bass
    import concourse.mybir as mybir
    import concourse.tile as tile

    f32 = mybir.dt.float32
    u8 = mybir.dt.uint8
    ts = bass.ts

    nc = bacc.Bacc("TRN2", target_bir_lowering=False, debug=False)

    queryT = nc.dram_tensor("queryT", [H, BPC], f32, kind="ExternalInput")[:]
    key = nc.dram_tensor("key", [BPC, S, H], f32, kind="ExternalInput")[:]
    value = nc.dram_tensor("value", [BPC, S, H], f32, kind="ExternalInput")[:]
    mask = nc.dram_tensor("mask", [BPC, S], u8, kind="ExternalInput")[:]
    WQT = nc.dram_tensor("WQT", [H, H], f32, kind="ExternalInput")[:]
    WK = nc.dram_tensor("WK", [H, H], f32, kind="ExternalInput")[:]
    WVT = nc.dram_tensor("WVT", [H, H], f32, kind="ExternalInput")[:]
    ident128 = nc.dram_tensor("ident128", [128, 128], f32, kind="ExternalInput")[:]
    out = nc.dram_tensor("out", [BPC, H], f32, kind="ExternalOutput")[:]
    pattn = nc.dram_tensor("pattn", [BPC, S], f32, kind="ExternalOutput")[:]

    with tile.TileContext(nc) as tc, ExitStack() as ctx:
        singles = ctx.enter_context(tc.tile_pool(name="singles", bufs=1))
        kpool = ctx.enter_context(tc.tile_pool(name="key", bufs=4))
        vpool = ctx.enter_context(tc.tile_pool(name="val", bufs=4))
        spool = ctx.enter_context(tc.tile_pool(name="small", bufs=3))
        jpool = ctx.enter_context(tc.tile_pool(name="junk", bufs=2))
        ps_u = ctx.enter_context(tc.tile_pool(name="ps_u", bufs=2, space="PSUM"))
        ps_t = ctx.enter_context(tc.tile_pool(name="ps_t", bufs=4, space="PSUM"))
        acc_pool = ctx.enter_context(tc.tile_pool(name="acc", bufs=2))

        # ---- constants ----
        ones_col = singles.tile([128, 1], f32)
        nc.vector.memset(ones_col[:], 1.0)
        ones_row = singles.tile([1, 128], f32)
        nc.vector.memset(ones_row[:], 1.0)
        ident = singles.tile([128, 128], f32)
        nc.sync.dma_start(ident[:], ident128)

        # ---- load weights / query (tiny) ----
        wqt = singles.tile([128, 2, H], f32)  # [d_p, dc, e] = WQ^T
        nc.sync.dma_start(wqt[:], WQT.rearrange("(dc p) e -> p dc e", p=128))
        wk = singles.tile([128, 2, H], f32)  # [e_p, ec, d] = WK (natural)
        nc.sync.dma_start(wk[:], WK.rearrange("(ec p) d -> p ec d", p=128))
        wvt = singles.tile([128, 2, H], f32)  # [d_p, dc, e] = WV^T
        nc.sync.dma_start(wvt[:], WVT.rearrange("(dc p) e -> p dc e", p=128))
        qt = singles.tile([128, 2, BPC], f32)  # [d_p, dc, b] = query^T
        nc.sync.dma_start(qt[:], queryT.rearrange("(dc p) b -> p dc b", p=128))

        # ---- q^T = WQ @ query^T : [e, b] ----
        q_sb = singles.tile([128, 2, BPC], f32)
        with tc.tile_pool(name="ps_setup", bufs=1, space="PSUM") as ps_set:
            for ec in range(2):
                qp = ps_set.tile([128, BPC], f32, tag="setup")
                for dc in range(2):
                    nc.tensor.matmul(
                        qp[:],
                        wqt[:, dc, ts(ec, 128)],
                        qt[:, dc, :],
                        start=(dc == 0),
                        stop=(dc == 1),
                    )
                nc.vector.tensor_copy(q_sb[:, ec, :], qp[:])
            # ---- q_tilde^T = (WK^T @ q^T) * SCALE : [d_p, dc, b] ----
            qtil = singles.tile([128, 2, BPC], f32)
            for dc in range(2):
                qtp = ps_set.tile([128, BPC], f32, tag="setup")
                for ec in range(2):
                    nc.tensor.matmul(
                        qtp[:],
                        wk[:, ec, ts(dc, 128)],
                        q_sb[:, ec, :],
                        start=(ec == 0),
                        stop=(ec == 1),
                    )
                nc.scalar.mul(qtil[:, dc, :], qtp[:], SCALE)

            # ---- broadcast every batch's q_tilde to all partitions ----
            # qball[:, b, :] = q_tilde_b replicated on 128 partitions
            qball = singles.tile([128, BPC, H], f32)
            for b in range(BPC):
                qrow = spool.tile([1, H], f32)
                for dc in range(2):
                    qrp = ps_t.tile([1, 128], f32, tag="tiny")
                    nc.tensor.transpose(qrp[:], qtil[:, dc, b : b + 1], ident[:])
                    nc.vector.tensor_copy(qrow[0:1, ts(dc, 128)], qrp[:])
                qbp = ps_set.tile([128, H], f32, tag="qb")
                nc.tensor.matmul(qbp[:], ones_row[:], qrow[:])
                nc.vector.tensor_copy(qball[:, b, :], qbp[:])

        # ---- per-batch attention ----
        HJ = SJ // 2  # half of the j range, for finer DMA/compute pipelining
        for b in range(BPC):
            key_b = key[b].rearrange("(p j) d -> p j d", p=128)
            val_b = value[b].rearrange("(p j) d -> p j d", p=128)
            kts = []
            vts = []
            for h in range(2):
                kt = kpool.tile([128, HJ, H], f32, tag="kt")  # s = p*32+j
                nc.sync.dma_start(kt[:], key_b[:, h * HJ : (h + 1) * HJ, :])
                kts.append(kt)
            for h in range(2):
                vt = vpool.tile([128, HJ, H], f32, tag="vt")
                nc.sync.dma_start(vt[:], val_b[:, h * HJ : (h + 1) * HJ, :])
                vts.append(vt)
            msk = spool.tile([128, SJ], u8)
            nc.sync.dma_start(msk[:], mask[b].rearrange("(p j) -> p j", p=128))
            mskf = spool.tile([128, SJ], f32)
            nc.vector.tensor_copy(mskf[:], msk[:])  # u8 -> f32 cast

            # Per half h: scores mult+reduce on DVE, mask+exp, then the
            # u-pass MMs for that half start while the other half's scores
            # are still being computed.  u' = e @ value (unnormalized);
            # normalization by 1/Z happens once at the end on u^T.
            scs = spool.tile([128, SJ], f32)
            e = spool.tile([128, SJ], f32)
            rs = spool.tile([128, 2], f32)
            mbig = spool.tile([128, SJ], f32)
            nc.scalar.mul(mbig[:], mskf[:], NEG)
            qb_bc = qball[:, b : b + 1, :].broadcast_to([128, HJ, H])
            up = ps_u.tile([1, H], f32, tag="uo")
            acc = acc_pool.tile([128, H], f32, tag="acc")
            tmp = acc_pool.tile([128, H], f32, tag="tmp")
            # j-columns the DVE handles instead of the PE (load balance)
            DVEJ = (14, 15, 29, 30, 31)
            pe_js = [j for j in range(SJ) if j not in DVEJ]
            for h in range(2):
                junk = jpool.tile([128, HJ, H], f32, tag="junk")
                nc.vector.tensor_tensor(
                    junk[:], kts[h][:], qb_bc, op=mybir.AluOpType.mult
                )
                sl = slice(h * HJ, (h + 1) * HJ)
                nc.vector.reduce_sum(
                    scs[:, sl].rearrange("p (j one) -> p j one", one=1),
                    junk[:],
                    axis=mybir.AxisListType.X,
                )
                nc.vector.tensor_tensor(
                    e[:, sl], scs[:, sl], mbig[:, sl], op=mybir.AluOpType.add
                )
                nc.scalar.activation(
                    e[:, sl],
                    e[:, sl],
                    mybir.ActivationFunctionType.Exp,
                    accum_out=rs[:, h : h + 1],
                )
                # PE share of u' for this half
                hjs = [j for j in pe_js if j // HJ == h]
                for j in hjs:
                    nc.tensor.matmul(
                        up[:],
                        e[:, j : j + 1],
                        vts[h][:, j % HJ, :],
                        start=(j == pe_js[0]),
                        stop=(j == pe_js[-1]),
                    )
                # DVE share of u' for this half
                for j in [j for j in DVEJ if j // HJ == h]:
                    vtj = vts[h][:, j % HJ, :]
                    if j == DVEJ[0]:
                        nc.vector.tensor_scalar_mul(acc[:], vtj, e[:, j : j + 1])
                    else:
                        nc.vector.tensor_scalar_mul(tmp[:], vtj, e[:, j : j + 1])
                        nc.vector.tensor_tensor(
                            acc[:], acc[:], tmp[:], op=mybir.AluOpType.add
                        )

            # Z = rs[:,0]+rs[:,1] summed over partitions; zi = 1/Z
            zp = ps_t.tile([1, 1], f32, tag="tiny")
            for h in range(2):
                nc.tensor.matmul(
                    zp[:], rs[:, h : h + 1], ones_col[:],
                    start=(h == 0), stop=(h == 1),
                )
            zi = spool.tile([1, 1], f32)
            nc.vector.reciprocal(zi[:], zp[:])
            zbp = ps_t.tile([128, 1], f32, tag="tiny")
            nc.tensor.matmul(zbp[:], ones_row[:], zi[:])
            zb = spool.tile([128, 1], f32)
            nc.vector.tensor_copy(zb[:], zbp[:])

            # p_attn = e * (1/Z) — off the critical path, on the ScalarE
            probs = spool.tile([128, SJ], f32)
            nc.scalar.mul(probs[:], e[:], zb[:])
            nc.scalar.dma_start(pattn[b].rearrange("(p j) -> p j", p=128), probs[:])

            # combine the two u' partials as u^T [128, 2], normalize, project
            up_sb = spool.tile([1, H], f32)
            nc.vector.tensor_copy(up_sb[:], up[:])
            ut = spool.tile([128, 2], f32)
            for dc in range(2):
                utp = ps_t.tile([128, 1], f32, tag="tiny")
                nc.tensor.transpose(utp[:], up_sb[0:1, ts(dc, 128)], ident[0:1, 0:1])
                nc.vector.tensor_copy(ut[:, dc : dc + 1], utp[:])
            utd = spool.tile([128, 2], f32)
            for dc in range(2):
                udp = ps_t.tile([128, 1], f32, tag="tiny")
                nc.tensor.matmul(udp[:], acc[:, ts(dc, 128)], ones_col[:])
                nc.vector.tensor_copy(utd[:, dc : dc + 1], udp[:])
            nc.vector.tensor_tensor(ut[:], ut[:], utd[:], op=mybir.AluOpType.add)
            nc.vector.tensor_scalar_mul(ut[:], ut[:], zb[:])
            op_ = ps_u.tile([1, H], f32, tag="uo")
            for dc in range(2):
                nc.tensor.matmul(
                    op_[:],
                    ut[:, dc : dc + 1],
                    wvt[:, dc, :],
                    start=(dc == 0),
                    stop=(dc == 1),
                )
            ob = spool.tile([1, H], f32)
            nc.scalar.copy(ob[:], op_[:])
            nc.scalar.dma_start(out[b : b + 1, :], ob[:])

    nc.compile()
    return nc


def _get_nc():
    if "nc" not in _NC_CACHE:
        _NC_CACHE["nc"] = build_nc()
    return _NC_CACHE["nc"]


def make_in_maps(query, key, value, mask):
    """Shard + lay out the full inputs for the 8 cores (no arithmetic)."""
    query = np.asarray(query, dtype=np.float32).reshape(B, H)
    key = np.asarray(key, dtype=np.float32)
    value = np.asarray(value, dtype=np.float32)
    mask_u8 = np.ascontiguousarray(np.asarray(mask)).view(np.uint8)
    ident = np.eye(128, dtype=np.float32)

    in_maps = []
    for c in range(NCORES):
        sl = slice(c * BPC, (c + 1) * BPC)
        q = query[sl]  # [BPC, H]
        in_maps.append(
            {
                "queryT": np.ascontiguousarray(q.T),
                "key": np.ascontiguousarray(key[sl]),
                "value": np.ascontiguousarray(value[sl]),
                "mask": np.ascontiguousarray(mask_u8[sl]),
                "ident128": ident,
            }
        )
    return in_maps


def _add_weights(in_maps, WQ, WK, WV):
    WQT = np.ascontiguousarray(np.asarray(WQ, dtype=np.float32).T)
    WKc = np.ascontiguousarray(np.asarray(WK, dtype=np.float32))
    WVT = np.ascontiguousarray(np.asarray(WV, dtype=np.float32).T)
    for m in in_maps:
        m["WQT"] = WQT
        m["WK"] = WKc
        m["WVT"] = WVT
    return in_maps


def run(query, key, value, mask, WQ, WK, WV, trace=False, **spmd_kwargs):
    from concourse.bass_utils import run_bass_kernel_spmd

    nc = _get_nc()
    in_maps = _add_weights(make_in_maps(query, key, value, mask), WQ, WK, WV)
    res = run_bass_kernel_spmd(
        nc, in_maps, list(range(NCORES)), trace=trace, **spmd_kwargs
    )
    outs = np.concatenate([res.results[c]["out"] for c in range(NCORES)], axis=0)
    patt = np.concatenate([res.results[c]["pattn"] for c in range(NCORES)], axis=0)
    return outs.reshape(B, 1, H), patt.reshape(B, 1, S), res


def kernel(query, key, value, mask, WQ, WK, WV):
    outs, patt, _ = run(query, key, value, mask, WQ, WK, WV)
    return outs, patt


# revision 22
# speedup vs baseline: 1.2789x; 1.0699x over previous
"""Trainium2 Bass kernel for dot-product attention with q_len=1.

Reference computation (per batch b):
    q = query @ WQ^T            [1, H]
    k = key @ WK^T              [S, H]
    v = value @ WV^T            [S, H]
    scores = q @ k^T / sqrt(H)  [1, S]
    scores = where(mask, -1e15, scores)
    p = softmax(scores)         [1, S]
    out = p @ v                 [1, H]
    returns (out, p)

Algebraic restructuring used here (exact same math, fp32 throughout):
    scores = (query @ WQ^T @ WK) @ key^T / sqrt(H)  -> fold both projections
             into a single per-batch vector q_tilde, never materialize k.
    out    = (p @ value) @ WV^T                     -> never materialize v.
This removes the two [S,H]x[H,H] GEMMs entirely; the kernel is then purely
memory-bound on streaming key+value (64 MiB/core) once, which is the roofline.

Work split on-chip: scores are computed on the VectorE (one big elementwise
multiply against the partition-broadcast q_tilde, then a 3D free-dim reduce;
key stays in its natural [s, d] layout), while the p@value contraction runs
on the TensorE (contraction along partitions).  fp32 matmuls cost 4 cycles/row on the PE,
so keeping the big key contraction off the PE is what makes both engines
fit under the DMA roofline.

Distribution: data-parallel over the batch dim, 8 batches per core on
8 cores.  The s index maps to (partition p, column j) as s = p*32 + j so
key/value/mask/p_attn transfers are all fully contiguous per partition.
"""

from contextlib import ExitStack

import numpy as np

B, S, H = 64, 4096, 256
NCORES = 8
BPC = B // NCORES  # batches per core
SJ = 32  # s-chunk columns: s = p*32 + j, p in [0,128), j in [0,32)
SCALE = 1.0 / 16.0  # 1/sqrt(H)
NEG = -1.0e15

_NC_CACHE = {}


def build_nc():
    import concourse.bacc as bacc
    import concourse.bass as bass
    import concourse.mybir as mybir
    import concourse.tile as tile

    f32 = mybir.dt.float32
    u8 = mybir.dt.uint8
    ts = bass.ts

    nc = bacc.Bacc("TRN2", target_bir_lowering=False, debug=False)

    queryT = nc.dram_tensor("queryT", [H, BPC], f32, kind="ExternalInput")[:]
    key = nc.dram_tensor("key", [BPC, S, H], f32, kind="ExternalInput")[:]
    value = nc.dram_tensor("value", [BPC, S, H], f32, kind="ExternalInput")[:]
    mask = nc.dram_tensor("mask", [BPC, S], u8, kind="ExternalInput")[:]
    WQT = nc.dram_tensor("WQT", [H, H], f32, kind="ExternalInput")[:]
    WK = nc.dram_tensor("WK", [H, H], f32, kind="ExternalInput")[:]
    WVT = nc.dram_tensor("WVT", [H, H], f32, kind="ExternalInput")[:]
    ident128 = nc.dram_tensor("ident128", [128, 128], f32, kind="ExternalInput")[:]
    out = nc.dram_tensor("out", [BPC, H], f32, kind="ExternalOutput")[:]
    pattn = nc.dram_tensor("pattn", [BPC, S], f32, kind="ExternalOutput")[:]

    with tile.TileContext(nc) as tc, ExitStack() as ctx:
        singles = ctx.enter_context(tc.tile_pool(name="singles", bufs=1))
        kpool = ctx.enter_context(tc.tile_pool(name="key", bufs=4))
        vpool = ctx.enter_context(tc.tile_pool(name="val", bufs=4))
        spool = ctx.enter_context(tc.tile_pool(name="small", bufs=3))
        jpool = ctx.enter_context(tc.tile_pool(name="junk", bufs=2))
        ps_u = ctx.enter_context(tc.tile_pool(name="ps_u", bufs=2, space="PSUM"))
        ps_qb = ctx.enter_context(tc.tile_pool(name="ps_qb", bufs=2, space="PSUM"))
        ps_t = ctx.enter_context(tc.tile_pool(name="ps_t", bufs=2, space="PSUM"))

        # ---- constants ----
        ones_col = singles.tile([128, 1], f32)
        nc.vector.memset(ones_col[:], 1.0)
        ones_row = singles.tile([1, 128], f32)
        nc.vector.memset(ones_row[:], 1.0)
        ident = singles.tile([128, 128], f32)
        nc.sync.dma_start(ident[:], ident128)

        # ---- load weights / query (tiny) ----
        wqt = singles.tile([128, 2, H], f32)  # [d_p, dc, e] = WQ^T
        nc.sync.dma_start(wqt[:], WQT.rearrange("(dc p) e -> p dc e", p=128))
        wk = singles.tile([128, 2, H], f32)  # [e_p, ec, d] = WK (natural)
        nc.sync.dma_start(wk[:], WK.rearrange("(ec p) d -> p ec d", p=128))
        wvt = singles.tile([128, 2, H], f32)  # [d_p, dc, e] = WV^T
        nc.sync.dma_start(wvt[:], WVT.rearrange("(dc p) e -> p dc e", p=128))
        qt = singles.tile([128, 2, BPC], f32)  # [d_p, dc, b] = query^T
        nc.sync.dma_start(qt[:], queryT.rearrange("(dc p) b -> p dc b", p=128))

        # ---- q^T = WQ @ query^T : [e, b] ----
        q_sb = singles.tile([128, 2, BPC], f32)
        with tc.tile_pool(name="ps_setup", bufs=1, space="PSUM") as ps_set:
            for ec in range(2):
                qp = ps_set.tile([128, BPC], f32, tag="setup")
                for dc in range(2):
                    nc.tensor.matmul(
                        qp[:],
                        wqt[:, dc, ts(ec, 128)],
                        qt[:, dc, :],
                        start=(dc == 0),
                        stop=(dc == 1),
                    )
                nc.vector.tensor_copy(q_sb[:, ec, :], qp[:])
            # ---- q_tilde^T = (WK^T @ q^T) * SCALE : [d_p, dc, b] ----
            qtil = singles.tile([128, 2, BPC], f32)
            for dc in range(2):
                qtp = ps_set.tile([128, BPC], f32, tag="setup")
                for ec in range(2):
                    nc.tensor.matmul(
                        qtp[:],
                        wk[:, ec, ts(dc, 128)],
                        q_sb[:, ec, :],
                        start=(ec == 0),
                        stop=(ec == 1),
                    )
                nc.scalar.mul(qtil[:, dc, :], qtp[:], SCALE)

            # ---- broadcast every batch's q_tilde to all partitions ----
            # qball[:, b, :] = q_tilde_b replicated on 128 partitions
            qball = singles.tile([128, BPC, H], f32)
            for b in range(BPC):
                qrow = spool.tile([1, H], f32)
                for dc in range(2):
                    qrp = ps_t.tile([1, 128], f32, tag="tiny")
                    nc.tensor.transpose(qrp[:], qtil[:, dc, b : b + 1], ident[:])
                    nc.vector.tensor_copy(qrow[0:1, ts(dc, 128)], qrp[:])
                qbp = ps_qb.tile([128, H], f32)
                nc.tensor.matmul(qbp[:], ones_row[:], qrow[:])
                nc.vector.tensor_copy(qball[:, b, :], qbp[:])

        # ---- per-batch attention ----
        HJ = SJ // 2  # half of the j range, for finer DMA/compute pipelining
        for b in range(BPC):
            key_b = key[b].rearrange("(p j) d -> p j d", p=128)
            val_b = value[b].rearrange("(p j) d -> p j d", p=128)
            kts = []
            vts = []
            for h in range(2):
                kt = kpool.tile([128, HJ, H], f32, tag="kt")  # s = p*32+j
                nc.sync.dma_start(kt[:], key_b[:, h * HJ : (h + 1) * HJ, :])
                kts.append(kt)
            for h in range(2):
                vt = vpool.tile([128, HJ, H], f32, tag="vt")
                nc.sync.dma_start(vt[:], val_b[:, h * HJ : (h + 1) * HJ, :])
                vts.append(vt)
            msk = spool.tile([128, SJ], u8)
            nc.sync.dma_start(msk[:], mask[b].rearrange("(p j) -> p j", p=128))
            mskf = spool.tile([128, SJ], f32)
            nc.vector.tensor_copy(mskf[:], msk[:])  # u8 -> f32 cast

            # Per half h: scores (DVE mult + reduce), mask-add, exp — then
            # that half's u-pass matmuls start on the PE while the DVE works
            # on the other half.  u' = e @ value is unnormalized; the 1/Z
            # scale is applied once afterwards.
            scs = spool.tile([128, SJ], f32)
            e = spool.tile([128, SJ], f32)
            rs = spool.tile([128, 2], f32)
            mbig = spool.tile([128, SJ], f32)
            nc.scalar.mul(mbig[:], mskf[:], NEG)
            qb_bc = qball[:, b : b + 1, :].broadcast_to([128, HJ, H])
            up = ps_u.tile([1, H], f32, tag="uo")
            for h in range(2):
                sl = slice(h * HJ, (h + 1) * HJ)
                junk = jpool.tile([128, HJ, H], f32, tag="junk")
                nc.vector.tensor_tensor(
                    junk[:], kts[h][:], qb_bc, op=mybir.AluOpType.mult
                )
                nc.vector.reduce_sum(
                    scs[:, sl].rearrange("p (j one) -> p j one", one=1),
                    junk[:],
                    axis=mybir.AxisListType.X,
                )
                nc.vector.tensor_tensor(
                    e[:, sl], scs[:, sl], mbig[:, sl], op=mybir.AluOpType.add
                )
                nc.scalar.activation(
                    e[:, sl],
                    e[:, sl],
                    mybir.ActivationFunctionType.Exp,
                    accum_out=rs[:, h : h + 1],
                )
                for j in range(h * HJ, (h + 1) * HJ):
                    nc.tensor.matmul(
                        up[:],
                        e[:, j : j + 1],
                        vts[h][:, j % HJ, :],
                        start=(j == 0),
                        stop=(j == SJ - 1),
                    )

            # Z = (rs0 + rs1) summed over partitions; zi = 1/Z
            zp = ps_t.tile([1, 1], f32, tag="tiny")
            for h in range(2):
                nc.tensor.matmul(
                    zp[:], rs[:, h : h + 1], ones_col[:],
                    start=(h == 0), stop=(h == 1),
                )
            zi = spool.tile([1, 1], f32)
            nc.vector.reciprocal(zi[:], zp[:])
            u = spool.tile([1, H], f32)
            nc.vector.tensor_scalar_mul(u[:], up[:], zi[:])

            # p_attn = e * (1/Z) broadcast — off the critical path
            zbp = ps_t.tile([128, 1], f32, tag="tiny")
            nc.tensor.matmul(zbp[:], ones_row[:], zi[:])
            zb = spool.tile([128, 1], f32)
            nc.vector.tensor_copy(zb[:], zbp[:])
            probs = spool.tile([128, SJ], f32)
            nc.vector.tensor_scalar_mul(probs[:], e[:], zb[:])
            nc.scalar.dma_start(pattn[b].rearrange("(p j) -> p j", p=128), probs[:])

            # u^T via TensorE transpose, then out = u @ WV^T
            ut = spool.tile([128, 2], f32)
            for dc in range(2):
                utp = ps_t.tile([128, 1], f32, tag="tiny")
                nc.tensor.transpose(utp[:], u[0:1, ts(dc, 128)], ident[0:1, 0:1])
                nc.vector.tensor_copy(ut[:, dc : dc + 1], utp[:])
            op_ = ps_u.tile([1, H], f32, tag="uo")
            for dc in range(2):
                nc.tensor.matmul(
                    op_[:],
                    ut[:, dc : dc + 1],
                    wvt[:, dc, :],
                    start=(dc == 0),
                    stop=(dc == 1),
                )
            ob = spool.tile([1, H], f32)
            nc.vector.tensor_copy(ob[:], op_[:])
            nc.scalar.dma_start(out[b : b + 1, :], ob[:])

    nc.compile()
    return nc


def _get_nc():
    if "nc" not in _NC_CACHE:
        _NC_CACHE["nc"] = build_nc()
    return _NC_CACHE["nc"]


def make_in_maps(query, key, value, mask):
    """Shard + lay out the full inputs for the 8 cores (no arithmetic)."""
    query = np.asarray(query, dtype=np.float32).reshape(B, H)
    key = np.asarray(key, dtype=np.float32)
    value = np.asarray(value, dtype=np.float32)
    mask_u8 = np.ascontiguousarray(np.asarray(mask)).view(np.uint8)
    ident = np.eye(128, dtype=np.float32)

    in_maps = []
    for c in range(NCORES):
        sl = slice(c * BPC, (c + 1) * BPC)
        q = query[sl]  # [BPC, H]
        in_maps.append(
            {
                "queryT": np.ascontiguousarray(q.T),
                "key": np.ascontiguousarray(key[sl]),
                "value": np.ascontiguousarray(value[sl]),
                "mask": np.ascontiguousarray(mask_u8[sl]),
                "ident128": ident,
            }
        )
    return in_maps


def _add_weights(in_maps, WQ, WK, WV):
    WQT = np.ascontiguousarray(np.asarray(WQ, dtype=np.float32).T)
    WKc = np.ascontiguousarray(np.asarray(WK, dtype=np.float32))
    WVT = np.ascontiguousarray(np.asarray(WV, dtype=np.float32).T)
    for m in in_maps:
        m["WQT"] = WQT
        m["WK"] = WKc
        m["WVT"] = WVT
    return in_maps


def run(query, key, value, mask, WQ, WK, WV, trace=False, **spmd_kwargs):
    from concourse.bass_utils import run_bass_kernel_spmd

    nc = _get_nc()
    in_maps = _add_weights(make_in_maps(query, key, value, mask), WQ, WK, WV)
    res = run_bass_kernel_spmd(
        nc, in_maps, list(range(NCORES)), trace=trace, **spmd_kwargs
    )
    outs = np.concatenate([res.results[c]["out"] for c in range(NCORES)], axis=0)
    patt = np.concatenate([res.results[c]["pattn"] for c in range(NCORES)], axis=0)
    return outs.reshape(B, 1, H), patt.reshape(B, 1, S), res


def kernel(query, key, value, mask, WQ, WK, WV):
    outs, patt, _ = run(query, key, value, mask, WQ, WK, WV)
    return outs, patt
